# revision 12
# baseline (speedup 1.0000x reference)
"""Causal self-attention (GQA + RoPE) Trainium2 Bass kernel.

Sharding: 8 cores = data-parallel over batch (2) x tensor-parallel over heads (4).
Each core handles one batch element and 8 q-heads / 2 kv-heads, producing a
partial output projection (wo input-dim sharded); partials are summed on host.

Device pipeline per core (all matmuls fp32r = full-rate on TRN2 PE):
  Phase 1: QKV projections from x^T (weights-stationary), RoPE via
           permutation-matmul + DVE combine. Q^T spills to DRAM scratch,
           K^T / V stay resident in SBUF.
  Phase 2: per-head flash-style attention in transposed score layout
           S^T[k, q] (no probability transposes needed), exp on ACT,
           denominator via ones-matmul (partition sum + broadcast in one),
           output attn^T[d, q] accumulated in PSUM.
  Phase 3: output projection out[tok, e] with attn^T tiles as stationary.
"""

import os
import sys
import time

for _p in ("/opt/trn_rl_repo", "/root/.axon_site/_ro/trn_rl_repo"):
    if os.path.isdir(_p) and _p not in sys.path:
        sys.path.append(_p)

import numpy as np

B, S, E = 2, 2048, 4096
H, KV, D = 32, 8, 128
HQ, HKV = 8, 2              # per-core q heads / kv heads
DQ, DKV = HQ * D, HKV * D   # 1024, 256
N_CORES = 8
ROPE_BASE = 500000.0
TW = 512                    # token tile width
TC = S // TW                # 4 token tiles
ECH = E // 128              # 32 contraction chunks

_cache: dict = {}
last_exec_s: float | None = None
_DEBUG = False


def _build_nc():
    import concourse.mybir as mybir
    import concourse.tile as tile
    from concourse import bacc
    from contextlib import ExitStack

    dt = mybir.dt
    f32, f32r = dt.float32, dt.float32r
    Exp = mybir.ActivationFunctionType.Exp
    SCALE = float(1.0 / np.sqrt(D))

    nc = bacc.Bacc("TRN2", target_bir_lowering=False, debug=False,
                   enable_asserts=False, num_devices=N_CORES)

    xT = nc.dram_tensor("xT", [E, S], f32r, kind="ExternalInput").ap()
    wqkv = nc.dram_tensor("wqkv", [E, DQ + 2 * DKV], f32r, kind="ExternalInput").ap()
    woT = nc.dram_tensor("woT", [DQ, E], f32r, kind="ExternalInput").ap()
    cosT = nc.dram_tensor("cosT", [128, S], f32, kind="ExternalInput").ap()
    sinT = nc.dram_tensor("sinT", [128, S], f32, kind="ExternalInput").ap()
    mskT = nc.dram_tensor("mskT", [4, 128, 512], f32, kind="ExternalInput").ap()
    onesT = nc.dram_tensor("onesT", [128, 128], f32r, kind="ExternalInput").ap()
    permT = nc.dram_tensor("permT", [128, 128], f32r, kind="ExternalInput").ap()
    out = nc.dram_tensor("out", [S, E], f32, kind="ExternalOutput").ap()

    xT3 = xT.rearrange("(c p) t -> p c t", p=128)    # [128, 32, 2048]
    w3 = wqkv.rearrange("(c p) d -> p c d", p=128)   # [128, 32, 1536]
    woT3 = woT.rearrange("(c p) e -> p c e", p=128)  # [128, 8, 4096]

    with tile.TileContext(nc) as tc:
        with ExitStack() as outer:
            dram = outer.enter_context(tc.tile_pool(name="dram", bufs=1, space="DRAM"))
            qt_scr = dram.tile([HQ, 128, S], f32r, tag="qts")
            at_scr = dram.tile([S // 128, HQ, 128, 128], f32r, tag="ats")

            with ExitStack() as ph12:
                pers = ph12.enter_context(tc.tile_pool(name="pers", bufs=1))
                kt_sb = pers.tile([128, HKV, S], f32r, tag="kt")
                v_sb = pers.tile([128, S // 128, DKV], f32r, tag="v")
                msk_sb = pers.tile([128, 4, 512], f32, tag="msk")
                ones_sb = pers.tile([128, 128], f32r, tag="ones")
                perm_sb = pers.tile([128, 128], f32r, tag="perm")
                nc.sync.dma_start(msk_sb[:], mskT.rearrange("c p f -> p c f"))
                nc.sync.dma_start(ones_sb[:], onesT)
                nc.sync.dma_start(perm_sb[:], permT)

                # ---------------- Phase 1: projections + RoPE ----------------
                with ExitStack() as ph1:
                    csp = ph1.enter_context(tc.tile_pool(name="cs", bufs=1))
                    cos_sb = csp.tile([128, S], f32, tag="cos")
                    sin_sb = csp.tile([128, S], f32, tag="sin")
                    nc.sync.dma_start(cos_sb[:], cosT)
                    nc.sync.dma_start(sin_sb[:], sinT)

                    strip_p = ph1.enter_context(tc.tile_pool(name="strip", bufs=5))
                    wch_p = ph1.enter_context(tc.tile_pool(name="wch", bufs=6))
                    acc_p = ph1.enter_context(tc.tile_pool(name="acc", bufs=14))
                    ps1 = ph1.enter_context(tc.tile_pool(name="ps1", bufs=6, space="PSUM"))
                    tmp_p = ph1.enter_context(tc.tile_pool(name="rtmp", bufs=3))
                    rot_p = ph1.enter_context(tc.tile_pool(name="rot", bufs=3))

                    for t in range(TC):
                        ts_ = slice(t * TW, (t + 1) * TW)
                        accs = {}
                        for j in range(HQ):
                            accs[("q", j)] = acc_p.tile([128, TW], f32r, tag="acc", name=f"accq{j}")
                        for j in range(HKV):
                            accs[("k", j)] = acc_p.tile([128, TW], f32r, tag="acc", name=f"acck{j}")

                        for half in range(2):
                            first = half == 0
                            strips = []
                            for cc in range(4):
                                stile = strip_p.tile([128, 4, TW], f32r, tag="strip", name="stile")
                                c0 = half * 16 + cc * 4
                                nc.sync.dma_start(stile[:], xT3[:, c0:c0 + 4, ts_])
                                strips.append(stile)

                            def evac(dst, ps, first=first):
                                if first:
                                    nc.vector.tensor_copy(dst, ps)
                                else:
                                    nc.vector.tensor_add(dst, dst, ps)

                            # sets 0/1: q heads 0-3 / 4-7
                            for st in range(2):
                                pss = [ps1.tile([128, TW], f32, tag="ps", name=f"pss{_j}") for _j in range(4)]
                                for cc in range(4):
                                    c0 = half * 16 + cc * 4
                                    wt = wch_p.tile([128, 4, 512], f32r, tag="w", name="wt")
                                    nc.sync.dma_start(
                                        wt[:], w3[:, c0:c0 + 4, st * 512:(st + 1) * 512])
                                    for e in range(4):
                                        b0 = cc == 0 and e == 0
                                        bN = cc == 3 and e == 3
                                        for j in range(4):
                                            nc.tensor.matmul(
                                                pss[j][:],
                                                wt[:, e, j * 128:(j + 1) * 128],
                                                strips[cc][:, e, :],
                                                start=b0, stop=bN)
                                for j in range(4):
                                    evac(accs[("q", st * 4 + j)][:], pss[j][:])

                            # set 2: k0,k1 + v sub 0,1 then v sub 2,3 (reusing wt chunks)
                            psk = [ps1.tile([128, TW], f32, tag="ps", name=f"psk{_j}") for _j in range(2)]
                            psv = [ps1.tile([128, DKV], f32, tag="ps", name=f"psva{_j}") for _j in range(2)]
                            wts = []
                            for cc in range(4):
                                c0 = half * 16 + cc * 4
                                wt = wch_p.tile([128, 4, 512], f32r, tag="w", name="wt")
                                nc.sync.dma_start(wt[:], w3[:, c0:c0 + 4, 1024:1536])
                                wts.append(wt)
                                for e in range(4):
                                    b0 = cc == 0 and e == 0
                                    bN = cc == 3 and e == 3
                                    for j in range(2):
                                        nc.tensor.matmul(
                                            psk[j][:],
                                            wt[:, e, j * 128:(j + 1) * 128],
                                            strips[cc][:, e, :],
                                            start=b0, stop=bN)
                                    for sub in range(2):
                                        nc.tensor.matmul(
                                            psv[sub][:],
                                            strips[cc][:, e, sub * 128:(sub + 1) * 128],
                                            wt[:, e, 256:512],
                                            start=b0, stop=bN)
                            for j in range(2):
                                evac(accs[("k", j)][:], psk[j][:])
                            for sub in range(2):
                                evac(v_sb[:, t * 4 + sub, :], psv[sub][:])

                            psv2 = [ps1.tile([128, DKV], f32, tag="ps", name=f"psvb{_j}") for _j in range(2)]
                            for cc in range(4):
                                for e in range(4):
                                    b0 = cc == 0 and e == 0
                                    bN = cc == 3 and e == 3
                                    for sub in range(2):
                                        nc.tensor.matmul(
                                            psv2[sub][:],
                                            strips[cc][:, e, (2 + sub) * 128:(3 + sub) * 128],
                                            wts[cc][:, e, 256:512],
                                            start=b0, stop=bN)
                            for sub in range(2):
                                evac(v_sb[:, t * 4 + 2 + sub, :], psv2[sub][:])

                        # RoPE on q/k accumulators for this token tile
                        for key, a in accs.items():
                            sw = ps1.tile([128, TW], f32, tag="ps", name="sw")
                            nc.tensor.matmul(sw[:], perm_sb[:], a[:], start=True, stop=True)
                            t0 = tmp_p.tile([128, TW], f32, tag="t0")
                            nc.vector.tensor_mul(t0[:], a[:], cos_sb[:, ts_])
                            t1 = tmp_p.tile([128, TW], f32, tag="t1")
                            nc.vector.tensor_mul(t1[:], sw[:], sin_sb[:, ts_])
                            if key[0] == "q":
                                rot = rot_p.tile([128, TW], f32r, tag="rot")
                                nc.vector.tensor_add(rot[:], t0[:], t1[:])
                                nc.sync.dma_start(qt_scr[key[1], :, ts_], rot[:])
                            else:
                                nc.vector.tensor_add(kt_sb[:, key[1], ts_], t0[:], t1[:])

                if _DEBUG:
                    kt_dbg = nc.dram_tensor("kt_dbg", [128, HKV, S], f32r, kind="ExternalOutput").ap()
                    v_dbg = nc.dram_tensor("v_dbg", [128, S // 128, DKV], f32r, kind="ExternalOutput").ap()
                    qt_dbg = nc.dram_tensor("qt_dbg", [HQ, 128, S], f32r, kind="ExternalOutput").ap()
                    nc.sync.dma_start(kt_dbg, kt_sb[:])
                    nc.sync.dma_start(v_dbg, v_sb[:])
                    nc.sync.dma_start(qt_dbg, qt_scr[:])

                # ---------------- Phase 2: attention ----------------
                # wo first half preloads during phase 2; second half during phase 3.
                wo1p = outer.enter_context(tc.tile_pool(name="wo1", bufs=1, side="right"))
                wo1_sb = wo1p.tile([128, HQ, E // 2], f32r, tag="wo1")
                nc.sync.dma_start(wo1_sb[:], woT3[:, :, 0:E // 2])

                if _DEBUG:
                    es_dbg = nc.dram_tensor("es_dbg", [4, 128, TW], f32r, kind="ExternalOutput").ap()
                    da_dbg = nc.dram_tensor("da_dbg", [128, TW], f32r, kind="ExternalOutput").ap()
                    rc_dbg = nc.dram_tensor("rc_dbg", [128, TW], f32, kind="ExternalOutput").ap()
                    po_dbg = nc.dram_tensor("po_dbg", [128, TW], f32, kind="ExternalOutput").ap()

                with ExitStack() as ph2:
                    dbg_p = ph2.enter_context(tc.tile_pool(name="dbgp", bufs=2))
                    qt_p = ph2.enter_context(tc.tile_pool(name="qt", bufs=2))
                    sps_p = ph2.enter_context(tc.tile_pool(name="sps", bufs=3, space="PSUM"))
                    po_p = ph2.enter_context(tc.tile_pool(name="po", bufs=2, space="PSUM"))
                    mps_p = ph2.enter_context(tc.tile_pool(name="mps", bufs=2, space="PSUM"))
                    es_p = ph2.enter_context(tc.tile_pool(name="es", bufs=3))
                    da_p = ph2.enter_context(tc.tile_pool(name="da", bufs=2))
                    rc_p = ph2.enter_context(tc.tile_pool(name="rc", bufs=2))
                    at_p = ph2.enter_context(tc.tile_pool(name="at", bufs=2))

                    for h in range(HQ):
                        kv = h // 4
                        qt = qt_p.tile([128, S], f32r, tag="qt")
                        nc.sync.dma_start(qt[:], qt_scr[h])
                        for qc in range(4):
                            po = po_p.tile([128, TW], f32, tag="po", name="po")
                            da = da_p.tile([128, TW], f32r, tag="da")
                            nk = 4 * qc + 4
                            for kc in range(nk):
                                jd = kc - 4 * qc  # >=0 on the diagonal 512-block
                                sps = sps_p.tile([128, TW], f32, tag="sps", name="sps")
                                nc.tensor.matmul(
                                    sps[:], kt_sb[:, kv, kc * 128:(kc + 1) * 128],
                                    qt[:, qc * TW:(qc + 1) * TW], start=True, stop=True)
                                es = es_p.tile([128, TW], f32r, tag="es")
                                nc.scalar.activation(es[:], sps[:], Exp, scale=SCALE)
                                if jd >= 0:
                                    nc.vector.tensor_mul(es[:], es[:], msk_sb[:, jd, :])
                                if kc == 0:
                                    nc.vector.tensor_copy(da[:], es[:])
                                else:
                                    nc.vector.tensor_add(da[:], da[:], es[:])
                                if _DEBUG and h == 0 and qc == 0:
                                    nc.sync.dma_start(es_dbg[kc], es[:])
                                nc.tensor.matmul(
                                    po[:], v_sb[:, kc, kv * 128:(kv + 1) * 128], es[:],
                                    start=(kc == 0), stop=(kc == nk - 1))
                            db = mps_p.tile([128, TW], f32, tag="db", name="db")
                            nc.tensor.matmul(db[:], ones_sb[:], da[:], start=True, stop=True)
                            rc = rc_p.tile([128, TW], f32, tag="rc")
                            nc.vector.reciprocal(rc[:], db[:])
                            at = at_p.tile([128, TW], f32r, tag="at")
                            if _DEBUG and h == 0 and qc == 0:
                                nc.sync.dma_start(da_dbg, da[:])
                                nc.sync.dma_start(rc_dbg, rc[:])
                                po_sb = dbg_p.tile([128, TW], f32, tag="posb", name="po_sb")
                                nc.vector.tensor_copy(po_sb[:], po[:])
                                nc.sync.dma_start(po_dbg, po_sb[:])
                            nc.vector.tensor_mul(at[:], po[:], rc[:])
                            nc.sync.dma_start(
                                at_scr[4 * qc:4 * qc + 4, h].rearrange("c p f -> p c f"),
                                at.rearrange("p (c f) -> p c f", c=4))

            if _DEBUG:
                at_dbg = nc.dram_tensor("at_dbg", [S // 128, HQ, 128, 128], f32r, kind="ExternalOutput").ap()
                nc.sync.dma_start(at_dbg, at_scr[:])

            # ---------------- Phase 3: output projection ----------------
            with ExitStack() as ph3:
                wo2p = ph3.enter_context(tc.tile_pool(name="wo2", bufs=1))
                wo2_sb = wo2p.tile([128, HQ, E // 2], f32r, tag="wo2")
                nc.sync.dma_start(wo2_sb[:], woT3[:, :, E // 2:E])

                ain_p = ph3.enter_context(tc.tile_pool(name="ain", bufs=3))
                wop_ps = ph3.enter_context(tc.tile_pool(name="wops", bufs=8, space="PSUM"))
                ob_p = ph3.enter_context(tc.tile_pool(name="ob", bufs=2))

                for pas in range(2):
                    wo_half = wo1_sb if pas == 0 else wo2_sb
                    for t2 in range(S // 128):
                        ats = ain_p.tile([128, HQ, 128], f32r, tag="ain")
                        nc.sync.dma_start(ats[:], at_scr[t2].rearrange("h p f -> p h f"))
                        ob = ob_p.tile([128, E // 2], f32, tag="ob")
                        pws = [wop_ps.tile([128, 512], f32, tag="pws", name=f"pws{_j}") for _j in range(4)]
                        for hd in range(HQ):
                            for e4 in range(4):
                                nc.tensor.matmul(
                                    pws[e4][:], ats[:, hd, :],
                                    wo_half[:, hd, e4 * 512:(e4 + 1) * 512],
                                    start=(hd == 0), stop=(hd == HQ - 1))
                        for e4 in range(4):
                            dst = ob[:, e4 * 512:(e4 + 1) * 512]
                            if e4 % 2 == 0:
                                nc.scalar.copy(dst, pws[e4][:])
                            else:
                                nc.vector.tensor_copy(dst, pws[e4][:])
                        nc.sync.dma_start(
                            out[t2 * 128:(t2 + 1) * 128, pas * (E // 2):(pas + 1) * (E // 2)],
                            ob[:])

    nc.compile()
    return nc


def _get_runner():
    if "runner" in _cache:
        return _cache["runner"]
    import jax
    from jax.sharding import Mesh, PartitionSpec, NamedSharding
    from jax.experimental.shard_map import shard_map
    import concourse.mybir as mybir
    from concourse import bass2jax

    nc = _build_nc()
    bass2jax.install_neuronx_cc_hook()

    part_name = nc.partition_id_tensor.name if nc.partition_id_tensor else None
    in_names, out_names, out_avals = [], [], []
    for alloc in nc.m.functions[0].allocations:
        if not isinstance(alloc, mybir.MemoryLocationSet):
            continue
        name = alloc.memorylocations[0].name
        if alloc.kind == "ExternalInput":
            if name != part_name:
                in_names.append(name)
        elif alloc.kind == "ExternalOutput":
            out_names.append(name)
            out_avals.append(jax.core.ShapedArray(
                tuple(alloc.tensor_shape), mybir.dt.np(alloc.dtype)))
    n_params = len(in_names)
    all_in = list(in_names + out_names)
    if part_name is not None:
        all_in.append(part_name)
    all_in = tuple(all_in)

    def _body(*args):
        operands = list(args)
        if part_name is not None:
            operands.append(bass2jax.partition_id_tensor())
        outs = bass2jax._bass_exec_p.bind(
            *operands, out_avals=tuple(out_avals), in_names=all_in,
            out_names=tuple(out_names), lowering_input_output_aliases=(),
            sim_require_finite=True, sim_require_nnan=True, nc=nc)
        return tuple(outs)

    devices = jax.devices()[:N_CORES]
    mesh = Mesh(np.asarray(devices), ("core",))
    no = len(out_names)
    sharded = jax.jit(
        shard_map(_body, mesh=mesh,
                  in_specs=(PartitionSpec("core"),) * (n_params + no),
                  out_specs=(PartitionSpec("core"),) * no, check_rep=False),
        donate_argnums=tuple(range(n_params, n_params + no)), keep_unused=True)
    sharding = NamedSharding(mesh, PartitionSpec("core"))
    runner = dict(sharded=sharded, in_names=in_names, out_names=out_names,
                  out_avals=out_avals, sharding=sharding, jax=jax)
    _cache["runner"] = runner
    return runner


def _run(in_maps):
    global last_exec_s
    r = _get_runner()
    jax = r["jax"]
    concat_in = [np.concatenate([np.ascontiguousarray(m[n]) for m in in_maps], axis=0)
                 for n in r["in_names"]]
    dev_in = [jax.device_put(a, r["sharding"]) for a in concat_in]
    zeros = [jax.device_put(
        np.zeros((N_CORES * a.shape[0], *a.shape[1:]), a.dtype), r["sharding"])
        for a in r["out_avals"]]
    for z in zeros:
        z.block_until_ready()
    for d in dev_in:
        d.block_until_ready()
    t0 = time.perf_counter()
    out_arrs = r["sharded"](*dev_in, *zeros)
    out_arrs = [np.asarray(o) for o in out_arrs]
    last_exec_s = time.perf_counter() - t0
    return [
        {name: out_arrs[i].reshape(N_CORES, *r["out_avals"][i].shape)[c]
         for i, name in enumerate(r["out_names"])}
        for c in range(N_CORES)
    ]


def _host_inputs(x, wq, wk, wv, wo, input_pos):
    x = np.asarray(x, np.float32)
    wq = np.asarray(wq, np.float32)
    wk = np.asarray(wk, np.float32)
    wv = np.asarray(wv, np.float32)
    wo = np.asarray(wo, np.float32)
    pos = np.asarray(input_pos).astype(np.float32)

    half = D // 2
    inv = (1.0 / (ROPE_BASE ** (np.arange(half, dtype=np.float32) / half))).astype(np.float32)
    ang = pos[:, None] * inv[None, :]          # [S, 64] f32
    cos, sin = np.cos(ang), np.sin(ang)
    cosT = np.empty((128, S), np.float32)
    cosT[0:half] = cos.T
    cosT[half:] = cos.T
    sinT = np.empty((128, S), np.float32)
    sinT[0:half] = -sin.T
    sinT[half:] = sin.T
    # mask4[j, p, f] = 1 where (f - 128*j) >= p  (valid k <= q on diagonal block j)
    jj = np.arange(4)[:, None, None]
    pp = np.arange(128)[None, :, None]
    ff = np.arange(512)[None, None, :]
    mask4 = ((ff - 128 * jj) >= pp).astype(np.float32)
    ones = np.ones((128, 128), np.float32)
    perm = np.zeros((128, 128), np.float32)
    perm[(np.arange(128) + half) % 128, np.arange(128)] = 1.0

    in_maps = []
    for c in range(N_CORES):
        b, hg = c // 4, c % 4
        xTc = np.ascontiguousarray(x[b].T)
        wq_s = wq[hg * DQ:(hg + 1) * DQ]
        wk_s = wk[hg * DKV:(hg + 1) * DKV]
        wv_s = wv[hg * DKV:(hg + 1) * DKV]
        wqkv = np.ascontiguousarray(
            np.concatenate([wq_s.T, wk_s.T, wv_s.T], axis=1))
        woTs = np.ascontiguousarray(wo[:, hg * DQ:(hg + 1) * DQ].T)
        in_maps.append(dict(xT=xTc, wqkv=wqkv, woT=woTs, cosT=cosT, sinT=sinT,
                            mskT=mask4, onesT=ones, permT=perm))
    return in_maps


def kernel(x, wq, wk, wv, wo, input_pos):
    in_maps = _host_inputs(x, wq, wk, wv, wo, input_pos)
    results = _run(in_maps)
    out = np.zeros((B, S, E), np.float32)
    for c in range(N_CORES):
        out[c // 4] += results[c]["out"]
    return out
